# revision 46
# baseline (speedup 1.0000x reference)
"""MultiHeadAttention Trainium2 kernel.

Full inputs -> full output. Sharding: 8 cores = (batch b in 0..3) x (head
group hg in 0..1). Each core projects Q/K/V for its 8 heads (columns
hg*512..hg*512+512 of wq/wk/wv) over batch b's full 2048 rows, runs
attention for those heads, and applies its half of the output projection
(rows hg*512.. of wo). Host sums the two partial outputs per batch.

All matmuls in float32r (1 cycle/row for N>=256, ~2^-12 rounding).

Schedule: the Act engine's exp stream (~300us) is the critical path, so
all other PE work is injected INTO the attention loop as filler, with
each filler's input DMA issued several chunks ahead of its compute:

  prefix:  K^T projection for head pairs 0-1, Q^T (spilled to DRAM) for
           query blocks 0-3.
  B loop:  16 pair-iterations (head pair, 512-query block). Both heads'
           logits per key chunk are adjacent matmuls on disjoint PE row
           groups (base partitions 0/64) -> concurrent via row tiling.
           exp(0.125*logits + mask*(-1e9)) on Act; ctx matmuls (V carries
           a ones column -> softmax denominators) trail by LAG chunks.
           Fillers: V projection (iter 0), Q blocks 4-7 (iter 1), K^T
           pairs 2-3 (iters 2-3, xk re-streamed), normalize + output
           projection for query blocks 0-1 (iters 8-15).
  tail:    normalize + output projection for query blocks 2-3.
"""

import numpy as np

import concourse.bass as bass
import concourse.mybir as mybir
import concourse.tile as tile
from concourse import bacc
from concourse.bass_utils import run_bass_kernel_spmd

f32 = mybir.dt.float32
f32r = mybir.dt.float32r

B, S, D, H, DH = 4, 2048, 1024, 16, 64
HD2 = D // 2         # 512 columns per head group
N_CORES = 8
Exp = mybir.ActivationFunctionType.Exp
Ident = mybir.ActivationFunctionType.Identity

KC = D // 128        # 8 contraction chunks over the model dim
OC = HD2 // 128      # 4 output chunks (local head pairs)
SKC = S // 128       # 16 key chunks
XB = 8               # x streamed in 8 blocks of 256 seq positions
NT = 8 * 4           # 32 denominator rows (local head, query block)
NP = 16              # phase-B pair-iterations
LAG = 4              # ctx trails logits by LAG chunks within a pair-iter


def _iter_map():
    """Emission order of phase-B pair-iterations -> (head pair ko, query
    block). First half ko-major (K pairs 2-3 project as fillers in iters
    2-3); second half sqb-major (so sqb2's output projection can start at
    iter 12 and only sqb3 remains for the tail)."""
    seq = []
    for ko in range(OC):
        for sqs in range(2):
            seq.append((ko, sqs))
    for sqb in (2, 3):
        for ko in range(OC):
            seq.append((ko, sqb))
    return seq


def _build():
    nc = bacc.Bacc(None, target_bir_lowering=False)

    # pre-chunked host layouts (see kernel() below)
    xq = nc.dram_tensor("xq", [XB, 128, KC, 256], f32r, kind="ExternalInput")
    xk = nc.dram_tensor("xk", [XB, 128, KC, 256], f32r, kind="ExternalInput")
    xv = nc.dram_tensor("xv", [XB, 128, KC, 256], f32r, kind="ExternalInput")
    wq = nc.dram_tensor("wq", [128, KC, HD2], f32r, kind="ExternalInput")
    wk = nc.dram_tensor("wk", [128, KC, HD2], f32r, kind="ExternalInput")
    wv = nc.dram_tensor("wv", [128, KC, HD2], f32r, kind="ExternalInput")
    wo = nc.dram_tensor("wo", [128, OC, D], f32r, kind="ExternalInput")
    bq = nc.dram_tensor("bq", [128, OC], f32, kind="ExternalInput")
    bk = nc.dram_tensor("bk", [128, OC], f32, kind="ExternalInput")
    bo = nc.dram_tensor("bo", [128, D], f32, kind="ExternalInput")
    mb = nc.dram_tensor("mb", [128, SKC], f32, kind="ExternalInput")   # mask*-1e9
    sel = nc.dram_tensor("sel", [4, 8, OC, 128], f32r, kind="ExternalInput")
    out = nc.dram_tensor("out", [S, D], f32, kind="ExternalOutput")

    with tile.TileContext(nc) as tc:
        _emit(nc, tc, xq, xk, xv, wq, wk, wv, wo, bq, bk, bo, mb, sel, out)
    nc.finalize()
    return nc


def _emit(nc, tc, xq, xk, xv, wq, wk, wv, wo, bq, bk, bo, mb, sel, out):
    from contextlib import ExitStack

    with ExitStack() as ctx:
        consts = ctx.enter_context(tc.tile_pool(name="consts", bufs=1))
        kvres = ctx.enter_context(tc.tile_pool(name="kvres", bufs=1))
        wpool = ctx.enter_context(tc.tile_pool(name="wpool", bufs=2))
        xtp = ctx.enter_context(tc.tile_pool(name="xtp", bufs=3))
        qts = ctx.enter_context(tc.tile_pool(name="qts", bufs=2))
        ptp = ctx.enter_context(tc.tile_pool(name="ptp", bufs=5))
        stg = ctx.enter_context(tc.tile_pool(name="stg", bufs=2))
        stg2 = ctx.enter_context(tc.tile_pool(name="stg2", bufs=2))
        rbp = ctx.enter_context(tc.tile_pool(name="rbp", bufs=1))
        selp = ctx.enter_context(tc.tile_pool(name="selp", bufs=1))
        denp = ctx.enter_context(tc.tile_pool(name="denp", bufs=2))
        ctp = ctx.enter_context(tc.tile_pool(name="ctp", bufs=2))
        psA = ctx.enter_context(tc.tile_pool(name="psA", bufs=2, space="PSUM"))
        psC = ctx.enter_context(tc.tile_pool(name="psC", bufs=2, space="PSUM"))
        dram = ctx.enter_context(tc.tile_pool(name="dram", bufs=1, space="DRAM"))

        qtd = dram.tile([HD2, S], f32r)           # Q^T spill
        crd = dram.tile([OC, 128, S], f32r)       # raw ctx^T (out-proj layout)
        dnd = dram.tile([4, 8, 512], f32r)        # denominators [sqb, ko*2+h2]

        kT = kvres.tile([128, OC, S], f32r)          # K^T resident
        va = kvres.tile([128, SKC, 8, DH + 1], f32r)  # V + ones col resident
        nc.vector.memset(va[:, :, :, DH].bitcast(f32), 1.0)
        # allocated here, DMA'd after the startup-critical loads
        mb_sb = consts.tile([128, SKC], f32)
        bq_sb = consts.tile([128, OC], f32)
        bk_sb = consts.tile([128, OC], f32)

        # ---------- helper units: (prep, compute) pairs ----------
        def load_w(w_dram, name, split_first=False):
            w_sb = wpool.tile([128, KC, HD2], f32r, tag="w", name=name)
            if split_first:
                nc.sync.dma_start(w_sb[:, 0, :], w_dram[:, 0, :])
                nc.sync.dma_start(
                    w_sb.rearrange("p a b -> p (a b)")[:, HD2:],
                    w_dram[:].rearrange("p a b -> p (a b)")[:, HD2:])
            else:
                nc.sync.dma_start(w_sb.rearrange("p a b -> p (a b)"),
                                  w_dram[:].rearrange("p a b -> p (a b)"))
            return w_sb

        def x_prep(x_dram, blk, name):
            xt = xtp.tile([128, KC, 256], f32r, tag="xT", name=name)
            nc.sync.dma_start(xt, x_dram[blk])
            return xt

        def k_compute(wk_sb, xt, blk, dclo):
            ps = psA.tile([128, 3, 512], f32, tag="psA", name="psk")
            for dcl in range(2):
                dc = dclo + dcl
                for kc in range(KC):
                    nc.tensor.matmul(ps[:, dcl, 0:256],
                                     lhsT=wk_sb[:, kc, dc * 128:(dc + 1) * 128],
                                     rhs=xt[:, kc, :],
                                     start=(kc == 0), stop=(kc == KC - 1))
                nc.scalar.activation(
                    kT[:, dc, blk * 256:(blk + 1) * 256],
                    ps[:, dcl, 0:256], Ident, bias=bk_sb[:, dc:dc + 1])

        def q_compute(wq_sb, xt, blk):
            for dc2 in range(2):
                ps = psA.tile([128, 3, 512], f32, tag="psA", name="psq")
                stq = stg.tile([128, 2, 512], f32r, tag="stg", name="stq")
                for half in range(2):
                    dc = dc2 * 2 + half
                    for kc in range(KC):
                        nc.tensor.matmul(ps[:, half, 0:256],
                                         lhsT=wq_sb[:, kc, dc * 128:(dc + 1) * 128],
                                         rhs=xt[:, kc, :],
                                         start=(kc == 0), stop=(kc == KC - 1))
                    nc.scalar.activation(stq[:, half, 0:256], ps[:, half, 0:256],
                                         Ident, bias=bq_sb[:, dc:dc + 1])
                    nc.sync.dma_start(
                        qtd[dc * 128:(dc + 1) * 128, blk * 256:(blk + 1) * 256],
                        stq[:, half, 0:256])

        def v_compute(wv_sb, xt, blk):
            for sub in range(2):
                sc = blk * 2 + sub
                ps = psA.tile([128, 3, 512], f32, tag="psA", name="psv")
                for kc in range(KC):
                    nc.tensor.matmul(ps[:, 0, :],
                                     lhsT=xt[:, kc, sub * 128:(sub + 1) * 128],
                                     rhs=wv_sb[:, kc, :],
                                     start=(kc == 0), stop=(kc == KC - 1))
                with nc.allow_low_precision(reason="V rounded to f32r"):
                    nc.vector.tensor_copy(
                        va[:, sc, :, 0:DH],
                        ps[:, 0, :].rearrange("p (h d) -> p h d", h=8))

        cstate = {}

        def c_den_dma(sqb, lo=0, hi=8):
            if lo == 0:
                cstate["den%d" % sqb] = denp.tile([8, 512], f32r, tag="den",
                                                  name="den")
            nc.sync.dma_start(cstate["den%d" % sqb][lo:hi, :],
                              dnd[sqb, lo:hi, :])

        def c_den_fin(sqb):
            den_sb = cstate["den%d" % sqb]
            # tag "den": reuses the den ring slot (den is dead after recip)
            recf = denp.tile([8, 512], f32, tag="den", name="recf")
            nc.vector.reciprocal(recf, den_sb)
            rec = denp.tile([8, 512], f32r, tag="rec", name="rec")
            with nc.allow_low_precision(reason="softmax recip rounded to f32r"):
                nc.vector.tensor_copy(rec, recf)
            cstate["rec%d" % sqb] = rec

        def c_den_prep(sqb):
            c_den_dma(sqb)
            c_den_fin(sqb)

        def c_rb(sqb):
            sel_t = selp.tile([8, OC, 128], f32r, tag="sel", name="sel_t")
            nc.sync.dma_start(sel_t, sel[sqb])
            rbt = rbp.tile([128, OC, 512], f32r, tag="rb", name="rbt")
            for ko in range(OC):
                # psA (short-lived ring), NOT psC: a psC alloc here would
                # wait on a live ctx accumulator earlier in PE program
                # order -> deadlock
                pb = psA.tile([128, 3, 512], f32, tag="psA", name="pb")
                nc.tensor.matmul(pb[:, 0, :], lhsT=sel_t[:, ko, :],
                                 rhs=cstate["rec%d" % sqb][:],
                                 start=True, stop=True)
                with nc.allow_low_precision(reason="recip bcast in f32r"):
                    nc.vector.tensor_copy(rbt[:, ko, :], pb[:, 0, :])
            cstate["rbt"] = rbt

        def c_ld(sqb, kolo=0, kohi=OC):
            if kolo == 0:
                cstate["cT%d" % sqb] = ctp.tile([128, OC, 512], f32r,
                                                tag="cT", name="cT")
            nc.sync.dma_start(
                cstate["cT%d" % sqb][:, kolo:kohi, :],
                crd[kolo:kohi, :, sqb * 512:(sqb + 1) * 512]
                .rearrange("ko p q -> p ko q"))

        def c_norm(sqb):
            cT = cstate["cT%d" % sqb]
            with nc.allow_low_precision(reason="normalized ctx in f32r"):
                nc.vector.tensor_mul(out=cT, in0=cT, in1=cstate["rbt"])

        def c_st(st8):
            st4 = st8 % 4
            cT = cstate["cT%d" % (st8 // 4)]
            ps = psA.tile([128, 3, 512], f32, tag="psA", name="pso")
            for half in range(2):
                for ko in range(OC):
                    nc.tensor.matmul(
                        ps[:, half, :],
                        lhsT=cT[:, ko, st4 * 128:(st4 + 1) * 128],
                        rhs=cstate["wo"][:, ko, half * 512:(half + 1) * 512],
                        start=(ko == 0), stop=(ko == OC - 1))
            st_t = stg.tile([128, 2, 512], f32r, tag="stg", name="ost")
            with nc.allow_low_precision(reason="f32r storage is fp32 bits"):
                nc.vector.tensor_add(
                    out=st_t.rearrange("p a b -> p (a b)"),
                    in0=ps[:, 0:2, :].rearrange("p a b -> p (a b)"),
                    in1=cstate["bo"])
            nc.sync.dma_start(out[st8 * 128:(st8 + 1) * 128, :],
                              st_t.rearrange("p a b -> p (a b)").bitcast(f32))

        def late_wk2():
            cstate["wk2"] = load_w(wk, "wk2_sb", split_first=True)

        def load_wo():
            wo_raw = wpool.tile([128, KC, HD2], f32r, tag="w", name="wo_sb")
            nc.sync.dma_start(wo_raw.rearrange("p a b -> p (a b)"),
                              wo[:].rearrange("p a b -> p (a b)"))
            cstate["wo"] = wo_raw.rearrange("p (a c) b -> p a (c b)", c=2)
            bo_sb = consts.tile([128, D], f32)
            nc.sync.dma_start(bo_sb, bo[:])
            cstate["bo"] = bo_sb

        # ================= prefix =================
        # K pairs 0-1 and Q blocks 0-3, K/Q interleaved so both DMA streams
        # stay saturated
        wk_sb = load_w(wk, "wk_sb", split_first=True)
        wq_sb = load_w(wq, "wq_sb")
        xts = {}
        order = [("k", 0), ("k", 1), ("q", 0), ("k", 2), ("k", 3), ("q", 1),
                 ("k", 4), ("k", 5), ("q", 2), ("k", 6), ("k", 7), ("q", 3)]
        xts[order[0]] = x_prep(xk, 0, "xt")
        xts[order[1]] = x_prep(xk, 1, "xt")
        xts[order[2]] = x_prep(xq, 0, "xt")
        # mask/bias consts loaded after the startup-critical DMAs
        nc.sync.dma_start(mb_sb, mb[:])
        nc.sync.dma_start(bq_sb, bq[:])
        nc.sync.dma_start(bk_sb, bk[:])
        for n, (kind, blk) in enumerate(order):
            if n + 3 < len(order):
                k2, b2 = order[n + 3]
                xts[(k2, b2)] = x_prep(xk if k2 == "k" else xq, b2, "xt")
            if kind == "k":
                k_compute(wk_sb, xts.pop((kind, blk)), blk, 0)
            else:
                q_compute(wq_sb, xts.pop((kind, blk)), blk)
        wv_sb = load_w(wv, "wv_sb")   # consumed by V fillers in iter 0

        # ================= phase B with fillers =================
        # fill[(pair-iter, chunk)] -> list of closures; chunk -1 = boundary
        fill = {}

        def sched(i, sc, fn):
            fill.setdefault((i, max(sc, -1)), []).append(fn)

        # iter 0: V projection. prep(b) ~2 quanta ahead of compute(b)
        # (xtp bufs=3); compute(b) at chunk 2b+1 covers va chunks 2b,2b+1
        # (ctx deadline chunk 2b+LAG with LAG=5)
        for b in range(XB):
            sched(0, 2 * b - 4, lambda b=b: xts.__setitem__(
                ("v", b), x_prep(xv, b, "xvt")))
            sched(0, 2 * b + 1, lambda b=b: v_compute(wv_sb, xts.pop(("v", b)), b))
        # iter 1: Q blocks 4-7
        for j in range(4):
            sched(1, 4 * j - 7, lambda b=4 + j: xts.__setitem__(
                ("q", b), x_prep(xq, b, "xt")))
            sched(1, 4 * j + 1, lambda b=4 + j: q_compute(
                wq_sb, xts.pop(("q", b)), b))
        sched(1, 11, late_wk2)
        # iters 2-3: K pairs 2-3 (xk re-streamed)
        for b in range(XB):
            i_, j = divmod(b, 4)
            sched(2 + i_, 4 * j - 7, lambda b=b: xts.__setitem__(
                ("k2", b), x_prep(xk, b, "xt")))
            sched(2 + i_, 4 * j + 1, lambda b=b: k_compute(
                cstate["wk2"], xts.pop(("k2", b)), b, 2))
        sched(3, 13, load_wo)
        # iters 8-15: normalize + output projection for query blocks 0-2.
        # Each den chain (DMA -> recip -> cast) gets ~an iter before rb's
        # PE matmuls. Second-half iters are sqb-major: sqb2 rows complete
        # at iter 11, so only sqb3 is left for the tail.
        sched(8, 1, lambda: c_den_prep(0))
        sched(9, 1, lambda: c_rb(0))
        sched(9, 5, lambda: c_den_prep(1))
        sched(9, 9, lambda: c_ld(0))
        sched(9, 13, lambda: c_norm(0))
        for j in range(4):
            sched(10 + j, 1, lambda s=j: c_st(s))
        sched(10, 9, lambda: c_rb(1))
        sched(11, 9, lambda: c_ld(1))
        sched(11, 13, lambda: c_norm(1))
        sched(12, 5, lambda: c_den_prep(2))
        sched(13, 5, lambda: c_rb(2))
        sched(13, 9, lambda: c_ld(2))
        sched(13, 13, lambda: c_norm(2))
        st_slots = [(14, 1), (14, 9), (15, 1), (15, 9),
                    (14, 5), (14, 13), (15, 5), (15, 13)]
        for j in range(4):
            sched(*st_slots[j], lambda s=4 + j: c_st(s))
        for j in range(4):
            sched(*st_slots[4 + j], lambda s=8 + j: c_st(s))
        # tail prep hidden inside iter 15 (iter 15's own rows follow in tail)
        sched(15, 3, lambda: c_den_dma(3, 0, 6))
        sched(15, 11, lambda: c_ld(3, 0, 3))

        def run_fill(i, sc):
            for f in fill.get((i, sc), []):
                f()

        imap = _iter_map()
        qt_tiles = {}

        def load_qt(i):
            ko, sqb = imap[i]
            if (ko, sqb) not in qt_tiles:
                t = qts.tile([128, 512], f32r, tag="qt", name="qt")
                nc.sync.dma_start(
                    t, qtd[ko * 128:(ko + 1) * 128, sqb * 512:(sqb + 1) * 512])
                qt_tiles[(ko, sqb)] = t

        load_qt(0)
        for i in range(NP):
            ko, sqb = imap[i]
            sq2, sqs = divmod(sqb, 2)
            cur_qt = qt_tiles[(ko, sqb)]
            run_fill(i, -1)
            run_fill(i, -2)
            pts = []          # flat [128, 512] views, index f = 2*sc + h2
            pscs = [None, None]
            cur_psl = cur_pt = None

            def emit_ctx(sc):
                for h2 in range(2):
                    if sc == 0:
                        pscs[h2] = psC.tile([128, 512], f32, tag="psC",
                                            name="psc")
                    nc.tensor.matmul(pscs[h2][0:DH + 1, :],
                                     lhsT=va[:, sc, ko * 2 + h2, :],
                                     rhs=pts[2 * sc + h2],
                                     start=(sc == 0), stop=(sc == SKC - 1))

            for sc in range(SKC):
                # both heads' logits for key chunk sc — adjacent matmuls on
                # disjoint PE row groups (base partitions 0/64) run
                # concurrently (row tiling). Slots pack 3 per PSUM tile so
                # exp runs as [128, 1536] Act instructions.
                for h2 in range(2):
                    f = 2 * sc + h2
                    r = f % 3
                    if r == 0:
                        cur_psl = psA.tile([128, 3, 512], f32, tag="psA",
                                           name="psl")
                        cur_pt = ptp.tile([128, 3, 512], f32r, tag="pt",
                                          name="pt")
                    b0 = h2 * 64
                    nc.tensor.matmul(
                        cur_psl[:, r, :],
                        lhsT=kT[b0:b0 + 64, ko, sc * 128:(sc + 1) * 128],
                        rhs=cur_qt[b0:b0 + 64, :],
                        start=True, stop=True)
                    pts.append(cur_pt[:, r, :])
                    if r == 2 or f == 2 * SKC - 1:
                        w = r + 1
                        nc.scalar.activation(
                            cur_pt[:, 0:w, :].rearrange("p a b -> p (a b)"),
                            cur_psl[:, 0:w, :].rearrange("p a b -> p (a b)"),
                            Exp, bias=mb_sb[:, sc:sc + 1], scale=0.125)
                if sc >= LAG:
                    emit_ctx(sc - LAG)
                if sc == 10 and i + 1 < NP:
                    load_qt(i + 1)
                run_fill(i, sc)
            for sc in range(SKC - LAG, SKC):
                emit_ctx(sc)

            for h2 in range(2):
                h = ko * 2 + h2
                t = 2 * i + h2
                cu = stg2.tile([65, 512], f32r, tag="cu", name="cu")
                with nc.allow_low_precision(reason="raw ctx rounded to f32r"):
                    nc.vector.tensor_copy(cu, pscs[h2][0:DH + 1, :])
                nc.sync.dma_start(
                    crd[ko, h2 * 64:(h2 + 1) * 64, sqb * 512:(sqb + 1) * 512],
                    cu[0:DH, :])
                nc.sync.dma_start(dnd[sqb, 2 * ko + h2:2 * ko + h2 + 1, :],
                                  cu[DH:DH + 1, :])

        # ================= tail: sqb 3 =================
        c_den_dma(3, 6, 8)
        c_den_fin(3)
        c_rb(3)
        c_ld(3, 3, 4)
        c_norm(3)
        for j in range(4):
            c_st(12 + j)


_NC_CACHE = None


def _selector():
    # sel[sqb, rr, ko, p] = 1 iff denominator row dnd[sqb, rr] holds
    # (head ko*2 + p//64, query block sqb); rr = 2*ko + h2
    s = np.zeros((4, 8, OC, 128), np.float32)
    for sqb in range(4):
        for ko in range(OC):
            for h2 in range(2):
                s[sqb, 2 * ko + h2, ko, h2 * 64:(h2 + 1) * 64] = 1.0
    return s


def kernel(query, key, value, mask, wq, bq, wk, bk, wv, bv, wo, bo):
    global _NC_CACHE
    if _NC_CACHE is None:
        _NC_CACHE = _build()
    nc = _NC_CACHE

    query = np.asarray(query, dtype=np.float32)
    key = np.asarray(key, dtype=np.float32)
    value = np.asarray(value, dtype=np.float32)
    mask = np.asarray(mask, dtype=np.float32)
    wq_np = np.asarray(wq, np.float32)
    wk_np = np.asarray(wk, np.float32)
    wv_np = np.asarray(wv, np.float32)
    wo_np = np.asarray(wo, np.float32)
    bq_np = np.asarray(bq, np.float32)
    bk_np = np.asarray(bk, np.float32)
    bv_np = np.asarray(bv, np.float64)
    bo_np = np.asarray(bo, np.float64)

    def chunk_x(xT):
        # [1024, 2048] -> [XB, 128, KC, 256] with d = kc*128 + p
        return np.ascontiguousarray(
            xT.reshape(KC, 128, XB, 256).transpose(2, 1, 0, 3))

    def chunk_w(w_half):
        # [1024, 512] -> [128, KC, 512]
        return np.ascontiguousarray(
            w_half.reshape(KC, 128, HD2).transpose(1, 0, 2))

    xq_b, xk_b, xv_b = [], [], []
    for b in range(B):
        xq_b.append(chunk_x(np.ascontiguousarray(query[b].T)))
        xk_b.append(chunk_x(np.ascontiguousarray(key[b].T)))
        xv_b.append(chunk_x(np.ascontiguousarray(value[b].T)))

    sel_host = _selector()
    in_maps = []
    for core in range(N_CORES):
        b, hg = divmod(core, 2)
        sl = slice(hg * HD2, (hg + 1) * HD2)
        bias_out = bv_np[sl] @ wo_np[sl].astype(np.float64)
        if hg == 0:
            bias_out = bias_out + bo_np
        mbc = np.ascontiguousarray(
            (mask[b, 0, 0] * np.float32(-1e9)).reshape(SKC, 128).T)
        in_maps.append({
            "xq": xq_b[b], "xk": xk_b[b], "xv": xv_b[b],
            "wq": chunk_w(wq_np[:, sl]),
            "wk": chunk_w(wk_np[:, sl]),
            "wv": chunk_w(wv_np[:, sl]),
            "wo": np.ascontiguousarray(
                wo_np[sl].reshape(OC, 128, D).transpose(1, 0, 2)),
            "bq": np.ascontiguousarray(bq_np[sl].reshape(OC, 128).T),
            "bk": np.ascontiguousarray(bk_np[sl].reshape(OC, 128).T),
            "bo": np.ascontiguousarray(
                np.broadcast_to(bias_out.astype(np.float32), (128, D))),
            "mb": mbc, "sel": sel_host,
        })

    res = run_bass_kernel_spmd(nc, in_maps, core_ids=list(range(N_CORES)))
    full = np.empty((B, S, D), np.float32)
    for b in range(B):
        full[b] = res.results[2 * b]["out"]
        full[b] += res.results[2 * b + 1]["out"]
    return full


# revision 48
# speedup vs baseline: 1.0510x; 1.0510x over previous
"""MultiHeadAttention Trainium2 kernel.

Full inputs -> full output. Sharding: 8 cores = (batch b in 0..3) x (head
group hg in 0..1). Each core projects Q/K/V for its 8 heads (columns
hg*512..hg*512+512 of wq/wk/wv) over batch b's full 2048 rows, runs
attention for those heads, and applies its half of the output projection
(rows hg*512.. of wo). Host sums the two partial outputs per batch.

All matmuls in float32r (1 cycle/row for N>=256, ~2^-12 rounding).

Schedule: the Act engine's exp stream (~300us) is the critical path, so
all other PE work is injected INTO the attention loop as filler, with
each filler's input DMA issued several chunks ahead of its compute:

  prefix:  K^T projection for head pairs 0-1, Q^T (spilled to DRAM) for
           query blocks 0-3.
  B loop:  16 pair-iterations (head pair, 512-query block). Both heads'
           logits per key chunk are adjacent matmuls on disjoint PE row
           groups (base partitions 0/64) -> concurrent via row tiling.
           exp(0.125*logits + mask*(-1e9)) on Act; ctx matmuls (V carries
           a ones column -> softmax denominators) trail by LAG chunks.
           Fillers: V projection (iter 0), Q blocks 4-7 (iter 1), K^T
           pairs 2-3 (iters 2-3, xk re-streamed), normalize + output
           projection for query blocks 0-1 (iters 8-15).
  tail:    normalize + output projection for query blocks 2-3.
"""

import numpy as np

import concourse.bass as bass
import concourse.mybir as mybir
import concourse.tile as tile
from concourse import bacc
from concourse.bass_utils import run_bass_kernel_spmd

f32 = mybir.dt.float32
f32r = mybir.dt.float32r

B, S, D, H, DH = 4, 2048, 1024, 16, 64
HD2 = D // 2         # 512 columns per head group
N_CORES = 8
Exp = mybir.ActivationFunctionType.Exp
Ident = mybir.ActivationFunctionType.Identity

KC = D // 128        # 8 contraction chunks over the model dim
OC = HD2 // 128      # 4 output chunks (local head pairs)
SKC = S // 128       # 16 key chunks
XB = 8               # x streamed in 8 blocks of 256 seq positions
NT = 8 * 4           # 32 denominator rows (local head, query block)
NP = 16              # phase-B pair-iterations
LAG = 5              # ctx trails logits by LAG chunks within a pair-iter


def _iter_map():
    """Emission order of phase-B pair-iterations -> (head pair ko, query
    block). First half ko-major (K pairs 2-3 project as fillers in iters
    2-3); second half sqb-major (so sqb2's output projection can start at
    iter 12 and only sqb3 remains for the tail)."""
    seq = []
    for ko in range(OC):
        for sqs in range(2):
            seq.append((ko, sqs))
    for sqb in (2, 3):
        for ko in range(OC):
            seq.append((ko, sqb))
    return seq


def _build():
    nc = bacc.Bacc(None, target_bir_lowering=False)

    # pre-chunked host layouts (see kernel() below)
    xq = nc.dram_tensor("xq", [XB, 128, KC, 256], f32r, kind="ExternalInput")
    xk = nc.dram_tensor("xk", [XB, 128, KC, 256], f32r, kind="ExternalInput")
    xv = nc.dram_tensor("xv", [XB, 128, KC, 256], f32r, kind="ExternalInput")
    wq = nc.dram_tensor("wq", [128, KC, HD2], f32r, kind="ExternalInput")
    wk = nc.dram_tensor("wk", [128, KC, HD2], f32r, kind="ExternalInput")
    wv = nc.dram_tensor("wv", [128, KC, HD2], f32r, kind="ExternalInput")
    wo = nc.dram_tensor("wo", [128, OC, D], f32r, kind="ExternalInput")
    bq = nc.dram_tensor("bq", [128, OC], f32, kind="ExternalInput")
    bk = nc.dram_tensor("bk", [128, OC], f32, kind="ExternalInput")
    bo = nc.dram_tensor("bo", [128, D], f32, kind="ExternalInput")
    mb = nc.dram_tensor("mb", [128, SKC], f32, kind="ExternalInput")   # mask*-1e9
    sel = nc.dram_tensor("sel", [4, 8, OC, 128], f32r, kind="ExternalInput")
    out = nc.dram_tensor("out", [S, D], f32, kind="ExternalOutput")

    with tile.TileContext(nc) as tc:
        _emit(nc, tc, xq, xk, xv, wq, wk, wv, wo, bq, bk, bo, mb, sel, out)
    nc.finalize()
    return nc


def _emit(nc, tc, xq, xk, xv, wq, wk, wv, wo, bq, bk, bo, mb, sel, out):
    from contextlib import ExitStack

    with ExitStack() as ctx:
        consts = ctx.enter_context(tc.tile_pool(name="consts", bufs=1))
        kvres = ctx.enter_context(tc.tile_pool(name="kvres", bufs=1))
        wpool = ctx.enter_context(tc.tile_pool(name="wpool", bufs=2))
        xtp = ctx.enter_context(tc.tile_pool(name="xtp", bufs=3))
        qts = ctx.enter_context(tc.tile_pool(name="qts", bufs=2))
        ptp = ctx.enter_context(tc.tile_pool(name="ptp", bufs=7))
        stg = ctx.enter_context(tc.tile_pool(name="stg", bufs=2))
        stg2 = ctx.enter_context(tc.tile_pool(name="stg2", bufs=2))
        rbp = ctx.enter_context(tc.tile_pool(name="rbp", bufs=1))
        selp = ctx.enter_context(tc.tile_pool(name="selp", bufs=1))
        denp = ctx.enter_context(tc.tile_pool(name="denp", bufs=2))
        ctp = ctx.enter_context(tc.tile_pool(name="ctp", bufs=2))
        psA = ctx.enter_context(tc.tile_pool(name="psA", bufs=3, space="PSUM"))
        psC = ctx.enter_context(tc.tile_pool(name="psC", bufs=2, space="PSUM"))
        dram = ctx.enter_context(tc.tile_pool(name="dram", bufs=1, space="DRAM"))

        qtd = dram.tile([HD2, S], f32r)           # Q^T spill
        crd = dram.tile([OC, 128, S], f32r)       # raw ctx^T (out-proj layout)
        dnd = dram.tile([4, 8, 512], f32r)        # denominators [sqb, ko*2+h2]

        kT = kvres.tile([128, OC, S], f32r)          # K^T resident
        va = kvres.tile([128, SKC, 8, DH + 1], f32r)  # V + ones col resident
        nc.vector.memset(va[:, :, :, DH].bitcast(f32), 1.0)
        # allocated here, DMA'd after the startup-critical loads
        mb_sb = consts.tile([128, SKC], f32)
        bq_sb = consts.tile([128, OC], f32)
        bk_sb = consts.tile([128, OC], f32)

        # ---------- helper units: (prep, compute) pairs ----------
        def load_w(w_dram, name, split_first=False):
            w_sb = wpool.tile([128, KC, HD2], f32r, tag="w", name=name)
            if split_first:
                nc.sync.dma_start(w_sb[:, 0, :], w_dram[:, 0, :])
                nc.sync.dma_start(
                    w_sb.rearrange("p a b -> p (a b)")[:, HD2:],
                    w_dram[:].rearrange("p a b -> p (a b)")[:, HD2:])
            else:
                nc.sync.dma_start(w_sb.rearrange("p a b -> p (a b)"),
                                  w_dram[:].rearrange("p a b -> p (a b)"))
            return w_sb

        def x_prep(x_dram, blk, name):
            xt = xtp.tile([128, KC, 256], f32r, tag="xT", name=name)
            nc.sync.dma_start(xt, x_dram[blk])
            return xt

        def k_compute(wk_sb, xt, blk, dclo):
            ps = psA.tile([128, 2, 512], f32, tag="psA", name="psk")
            for dcl in range(2):
                dc = dclo + dcl
                for kc in range(KC):
                    nc.tensor.matmul(ps[:, dcl, 0:256],
                                     lhsT=wk_sb[:, kc, dc * 128:(dc + 1) * 128],
                                     rhs=xt[:, kc, :],
                                     start=(kc == 0), stop=(kc == KC - 1))
                nc.scalar.activation(
                    kT[:, dc, blk * 256:(blk + 1) * 256],
                    ps[:, dcl, 0:256], Ident, bias=bk_sb[:, dc:dc + 1])

        def q_compute(wq_sb, xt, blk):
            for dc2 in range(2):
                ps = psA.tile([128, 2, 512], f32, tag="psA", name="psq")
                stq = stg.tile([128, 2, 512], f32r, tag="stg", name="stq")
                for half in range(2):
                    dc = dc2 * 2 + half
                    for kc in range(KC):
                        nc.tensor.matmul(ps[:, half, 0:256],
                                         lhsT=wq_sb[:, kc, dc * 128:(dc + 1) * 128],
                                         rhs=xt[:, kc, :],
                                         start=(kc == 0), stop=(kc == KC - 1))
                    nc.scalar.activation(stq[:, half, 0:256], ps[:, half, 0:256],
                                         Ident, bias=bq_sb[:, dc:dc + 1])
                    nc.sync.dma_start(
                        qtd[dc * 128:(dc + 1) * 128, blk * 256:(blk + 1) * 256],
                        stq[:, half, 0:256])

        def v_compute(wv_sb, xt, blk):
            for sub in range(2):
                sc = blk * 2 + sub
                ps = psA.tile([128, 2, 512], f32, tag="psA", name="psv")
                for kc in range(KC):
                    nc.tensor.matmul(ps[:, 0, :],
                                     lhsT=xt[:, kc, sub * 128:(sub + 1) * 128],
                                     rhs=wv_sb[:, kc, :],
                                     start=(kc == 0), stop=(kc == KC - 1))
                with nc.allow_low_precision(reason="V rounded to f32r"):
                    nc.vector.tensor_copy(
                        va[:, sc, :, 0:DH],
                        ps[:, 0, :].rearrange("p (h d) -> p h d", h=8))

        cstate = {}

        def c_den_dma(sqb, lo=0, hi=8):
            if lo == 0:
                cstate["den%d" % sqb] = denp.tile([8, 512], f32r, tag="den",
                                                  name="den")
            nc.sync.dma_start(cstate["den%d" % sqb][lo:hi, :],
                              dnd[sqb, lo:hi, :])

        def c_den_fin(sqb):
            den_sb = cstate["den%d" % sqb]
            # tag "den": reuses the den ring slot (den is dead after recip)
            recf = denp.tile([8, 512], f32, tag="den", name="recf")
            nc.vector.reciprocal(recf, den_sb)
            rec = denp.tile([8, 512], f32r, tag="rec", name="rec")
            with nc.allow_low_precision(reason="softmax recip rounded to f32r"):
                nc.vector.tensor_copy(rec, recf)
            cstate["rec%d" % sqb] = rec

        def c_den_prep(sqb):
            c_den_dma(sqb)
            c_den_fin(sqb)

        def c_rb(sqb):
            sel_t = selp.tile([8, OC, 128], f32r, tag="sel", name="sel_t")
            nc.sync.dma_start(sel_t, sel[sqb])
            rbt = rbp.tile([128, OC, 512], f32r, tag="rb", name="rbt")
            for ko in range(OC):
                # psA (short-lived ring), NOT psC: a psC alloc here would
                # wait on a live ctx accumulator earlier in PE program
                # order -> deadlock
                pb = psA.tile([128, 2, 512], f32, tag="psA", name="pb")
                nc.tensor.matmul(pb[:, 0, :], lhsT=sel_t[:, ko, :],
                                 rhs=cstate["rec%d" % sqb][:],
                                 start=True, stop=True)
                with nc.allow_low_precision(reason="recip bcast in f32r"):
                    nc.vector.tensor_copy(rbt[:, ko, :], pb[:, 0, :])
            cstate["rbt"] = rbt

        def c_ld(sqb, kolo=0, kohi=OC):
            if kolo == 0:
                cstate["cT%d" % sqb] = ctp.tile([128, OC, 512], f32r,
                                                tag="cT", name="cT")
            nc.sync.dma_start(
                cstate["cT%d" % sqb][:, kolo:kohi, :],
                crd[kolo:kohi, :, sqb * 512:(sqb + 1) * 512]
                .rearrange("ko p q -> p ko q"))

        def c_norm(sqb):
            cT = cstate["cT%d" % sqb]
            with nc.allow_low_precision(reason="normalized ctx in f32r"):
                nc.vector.tensor_mul(out=cT, in0=cT, in1=cstate["rbt"])

        def c_st(st8):
            st4 = st8 % 4
            cT = cstate["cT%d" % (st8 // 4)]
            ps = psA.tile([128, 2, 512], f32, tag="psA", name="pso")
            for half in range(2):
                for ko in range(OC):
                    nc.tensor.matmul(
                        ps[:, half, :],
                        lhsT=cT[:, ko, st4 * 128:(st4 + 1) * 128],
                        rhs=cstate["wo"][:, ko, half * 512:(half + 1) * 512],
                        start=(ko == 0), stop=(ko == OC - 1))
            st_t = stg.tile([128, 2, 512], f32r, tag="stg", name="ost")
            with nc.allow_low_precision(reason="f32r storage is fp32 bits"):
                nc.vector.tensor_add(
                    out=st_t.rearrange("p a b -> p (a b)"),
                    in0=ps[:, 0:2, :].rearrange("p a b -> p (a b)"),
                    in1=cstate["bo"])
            nc.sync.dma_start(out[st8 * 128:(st8 + 1) * 128, :],
                              st_t.rearrange("p a b -> p (a b)").bitcast(f32))

        def late_wk2():
            cstate["wk2"] = load_w(wk, "wk2_sb", split_first=True)

        def load_wo():
            wo_raw = wpool.tile([128, KC, HD2], f32r, tag="w", name="wo_sb")
            nc.sync.dma_start(wo_raw.rearrange("p a b -> p (a b)"),
                              wo[:].rearrange("p a b -> p (a b)"))
            cstate["wo"] = wo_raw.rearrange("p (a c) b -> p a (c b)", c=2)
            bo_sb = consts.tile([128, D], f32)
            nc.sync.dma_start(bo_sb, bo[:])
            cstate["bo"] = bo_sb

        # ================= prefix =================
        # K pairs 0-1 and Q blocks 0-3, K/Q interleaved so both DMA streams
        # stay saturated
        wk_sb = load_w(wk, "wk_sb", split_first=True)
        wq_sb = load_w(wq, "wq_sb")
        xts = {}
        order = [("k", 0), ("k", 1), ("q", 0), ("k", 2), ("k", 3), ("q", 1),
                 ("k", 4), ("k", 5), ("q", 2), ("k", 6), ("k", 7), ("q", 3)]
        xts[order[0]] = x_prep(xk, 0, "xt")
        xts[order[1]] = x_prep(xk, 1, "xt")
        xts[order[2]] = x_prep(xq, 0, "xt")
        # mask/bias consts loaded after the startup-critical DMAs
        nc.sync.dma_start(mb_sb, mb[:])
        nc.sync.dma_start(bq_sb, bq[:])
        nc.sync.dma_start(bk_sb, bk[:])
        for n, (kind, blk) in enumerate(order):
            if n + 3 < len(order):
                k2, b2 = order[n + 3]
                xts[(k2, b2)] = x_prep(xk if k2 == "k" else xq, b2, "xt")
            if kind == "k":
                k_compute(wk_sb, xts.pop((kind, blk)), blk, 0)
            else:
                q_compute(wq_sb, xts.pop((kind, blk)), blk)
        wv_sb = load_w(wv, "wv_sb")   # consumed by V fillers in iter 0

        # ================= phase B with fillers =================
        # fill[(pair-iter, chunk)] -> list of closures; chunk -1 = boundary
        fill = {}

        def sched(i, sc, fn):
            fill.setdefault((i, max(sc, -1)), []).append(fn)

        # iter 0: V projection. prep(b) ~2 quanta ahead of compute(b)
        # (xtp bufs=3); compute(b) at chunk 2b+1 covers va chunks 2b,2b+1
        # (ctx deadline chunk 2b+LAG with LAG=5)
        for b in range(XB):
            sched(0, 2 * b - 4, lambda b=b: xts.__setitem__(
                ("v", b), x_prep(xv, b, "xvt")))
            sched(0, 2 * b + 1, lambda b=b: v_compute(wv_sb, xts.pop(("v", b)), b))
        # iter 1: Q blocks 4-7
        for j in range(4):
            sched(1, 4 * j - 7, lambda b=4 + j: xts.__setitem__(
                ("q", b), x_prep(xq, b, "xt")))
            sched(1, 4 * j + 1, lambda b=4 + j: q_compute(
                wq_sb, xts.pop(("q", b)), b))
        sched(1, 11, late_wk2)
        # iters 2-3: K pairs 2-3 (xk re-streamed)
        for b in range(XB):
            i_, j = divmod(b, 4)
            sched(2 + i_, 4 * j - 7, lambda b=b: xts.__setitem__(
                ("k2", b), x_prep(xk, b, "xt")))
            sched(2 + i_, 4 * j + 1, lambda b=b: k_compute(
                cstate["wk2"], xts.pop(("k2", b)), b, 2))
        sched(3, 13, load_wo)
        # iters 8-15: normalize + output projection for query blocks 0-2.
        # Each den chain (DMA -> recip -> cast) gets ~an iter before rb's
        # PE matmuls. Second-half iters are sqb-major: sqb2 rows complete
        # at iter 11, so only sqb3 is left for the tail.
        sched(8, 1, lambda: c_den_prep(0))
        sched(9, 1, lambda: c_rb(0))
        sched(9, 5, lambda: c_den_prep(1))
        sched(9, 9, lambda: c_ld(0))
        sched(9, 13, lambda: c_norm(0))
        for j in range(4):
            sched(10 + j, 1, lambda s=j: c_st(s))
        sched(10, 9, lambda: c_rb(1))
        sched(11, 9, lambda: c_ld(1))
        sched(11, 13, lambda: c_norm(1))
        sched(12, 5, lambda: c_den_prep(2))
        sched(13, 5, lambda: c_rb(2))
        sched(13, 9, lambda: c_ld(2))
        sched(13, 13, lambda: c_norm(2))
        st_slots = [(14, 1), (14, 9), (15, 1), (15, 9),
                    (14, 5), (14, 13), (15, 5), (15, 13)]
        for j in range(4):
            sched(*st_slots[j], lambda s=4 + j: c_st(s))
        for j in range(4):
            sched(*st_slots[4 + j], lambda s=8 + j: c_st(s))
        # tail prep hidden inside iter 15 (iter 15's own rows follow in tail)
        sched(15, 3, lambda: c_den_dma(3, 0, 6))
        sched(15, 11, lambda: c_ld(3, 0, 3))

        def run_fill(i, sc):
            for f in fill.get((i, sc), []):
                f()

        imap = _iter_map()
        qt_tiles = {}

        def load_qt(i):
            ko, sqb = imap[i]
            if (ko, sqb) not in qt_tiles:
                t = qts.tile([128, 512], f32r, tag="qt", name="qt")
                nc.sync.dma_start(
                    t, qtd[ko * 128:(ko + 1) * 128, sqb * 512:(sqb + 1) * 512])
                qt_tiles[(ko, sqb)] = t

        load_qt(0)
        for i in range(NP):
            ko, sqb = imap[i]
            sq2, sqs = divmod(sqb, 2)
            cur_qt = qt_tiles[(ko, sqb)]
            run_fill(i, -1)
            run_fill(i, -2)
            pts = []          # flat [128, 512] views, index f = 2*sc + h2
            pscs = [None, None]
            cur_psl = cur_pt = None

            def emit_ctx(sc):
                for h2 in range(2):
                    if sc == 0:
                        pscs[h2] = psC.tile([128, 512], f32, tag="psC",
                                            name="psc")
                    nc.tensor.matmul(pscs[h2][0:DH + 1, :],
                                     lhsT=va[:, sc, ko * 2 + h2, :],
                                     rhs=pts[2 * sc + h2],
                                     start=(sc == 0), stop=(sc == SKC - 1))

            for sc in range(SKC):
                # both heads' logits for key chunk sc — adjacent matmuls on
                # disjoint PE row groups (base partitions 0/64) run
                # concurrently (row tiling)
                cur_psl = psA.tile([128, 2, 512], f32, tag="psA", name="psl")
                cur_pt = ptp.tile([128, 2, 512], f32r, tag="pt", name="pt")
                for h2 in range(2):
                    b0 = h2 * 64
                    nc.tensor.matmul(
                        cur_psl[:, h2, :],
                        lhsT=kT[b0:b0 + 64, ko, sc * 128:(sc + 1) * 128],
                        rhs=cur_qt[b0:b0 + 64, :],
                        start=True, stop=True)
                    pts.append(cur_pt[:, h2, :])
                nc.scalar.activation(
                    cur_pt.rearrange("p a b -> p (a b)"),
                    cur_psl.rearrange("p a b -> p (a b)"), Exp,
                    bias=mb_sb[:, sc:sc + 1], scale=0.125)
                if sc >= LAG:
                    emit_ctx(sc - LAG)
                if sc == 10 and i + 1 < NP:
                    load_qt(i + 1)
                run_fill(i, sc)
            for sc in range(SKC - LAG, SKC):
                emit_ctx(sc)

            for h2 in range(2):
                h = ko * 2 + h2
                t = 2 * i + h2
                cu = stg2.tile([65, 512], f32r, tag="cu", name="cu")
                with nc.allow_low_precision(reason="raw ctx rounded to f32r"):
                    nc.vector.tensor_copy(cu, pscs[h2][0:DH + 1, :])
                nc.sync.dma_start(
                    crd[ko, h2 * 64:(h2 + 1) * 64, sqb * 512:(sqb + 1) * 512],
                    cu[0:DH, :])
                nc.sync.dma_start(dnd[sqb, 2 * ko + h2:2 * ko + h2 + 1, :],
                                  cu[DH:DH + 1, :])

        # ================= tail: sqb 3 =================
        c_den_dma(3, 6, 8)
        c_den_fin(3)
        c_rb(3)
        c_ld(3, 3, 4)
        c_norm(3)
        for j in range(4):
            c_st(12 + j)


_NC_CACHE = None


def _selector():
    # sel[sqb, rr, ko, p] = 1 iff denominator row dnd[sqb, rr] holds
    # (head ko*2 + p//64, query block sqb); rr = 2*ko + h2
    s = np.zeros((4, 8, OC, 128), np.float32)
    for sqb in range(4):
        for ko in range(OC):
            for h2 in range(2):
                s[sqb, 2 * ko + h2, ko, h2 * 64:(h2 + 1) * 64] = 1.0
    return s


def kernel(query, key, value, mask, wq, bq, wk, bk, wv, bv, wo, bo):
    global _NC_CACHE
    if _NC_CACHE is None:
        _NC_CACHE = _build()
    nc = _NC_CACHE

    query = np.asarray(query, dtype=np.float32)
    key = np.asarray(key, dtype=np.float32)
    value = np.asarray(value, dtype=np.float32)
    mask = np.asarray(mask, dtype=np.float32)
    wq_np = np.asarray(wq, np.float32)
    wk_np = np.asarray(wk, np.float32)
    wv_np = np.asarray(wv, np.float32)
    wo_np = np.asarray(wo, np.float32)
    bq_np = np.asarray(bq, np.float32)
    bk_np = np.asarray(bk, np.float32)
    bv_np = np.asarray(bv, np.float64)
    bo_np = np.asarray(bo, np.float64)

    def chunk_x(xT):
        # [1024, 2048] -> [XB, 128, KC, 256] with d = kc*128 + p
        return np.ascontiguousarray(
            xT.reshape(KC, 128, XB, 256).transpose(2, 1, 0, 3))

    def chunk_w(w_half):
        # [1024, 512] -> [128, KC, 512]
        return np.ascontiguousarray(
            w_half.reshape(KC, 128, HD2).transpose(1, 0, 2))

    xq_b, xk_b, xv_b = [], [], []
    for b in range(B):
        xq_b.append(chunk_x(np.ascontiguousarray(query[b].T)))
        xk_b.append(chunk_x(np.ascontiguousarray(key[b].T)))
        xv_b.append(chunk_x(np.ascontiguousarray(value[b].T)))

    sel_host = _selector()
    in_maps = []
    for core in range(N_CORES):
        b, hg = divmod(core, 2)
        sl = slice(hg * HD2, (hg + 1) * HD2)
        bias_out = bv_np[sl] @ wo_np[sl].astype(np.float64)
        if hg == 0:
            bias_out = bias_out + bo_np
        mbc = np.ascontiguousarray(
            (mask[b, 0, 0] * np.float32(-1e9)).reshape(SKC, 128).T)
        in_maps.append({
            "xq": xq_b[b], "xk": xk_b[b], "xv": xv_b[b],
            "wq": chunk_w(wq_np[:, sl]),
            "wk": chunk_w(wk_np[:, sl]),
            "wv": chunk_w(wv_np[:, sl]),
            "wo": np.ascontiguousarray(
                wo_np[sl].reshape(OC, 128, D).transpose(1, 0, 2)),
            "bq": np.ascontiguousarray(bq_np[sl].reshape(OC, 128).T),
            "bk": np.ascontiguousarray(bk_np[sl].reshape(OC, 128).T),
            "bo": np.ascontiguousarray(
                np.broadcast_to(bias_out.astype(np.float32), (128, D))),
            "mb": mbc, "sel": sel_host,
        })

    res = run_bass_kernel_spmd(nc, in_maps, core_ids=list(range(N_CORES)))
    full = np.empty((B, S, D), np.float32)
    for b in range(B):
        full[b] = res.results[2 * b]["out"]
        full[b] += res.results[2 * b + 1]["out"]
    return full
